# revision 7
# baseline (speedup 1.0000x reference)
"""GNN message-passing layer on 8 TRN2 NeuronCores (v4).

Reference computation (N=16384, D=128):
    a    = adj_mat.astype(f32)            # [N, N]
    deg  = a.sum(axis=0)                  # [N]
    agg  = (a^T @ x) / deg[:, None]       # [N, D]
    out  = relu(agg @ U^T)[None]          # [1, N, D]

Sharding: column-shard adj_mat across the 8 cores (core c owns output
nodes i in [c*2048, (c+1)*2048)); x and U replicated. The contraction
over j is fully local per core — no collective.

Host staging (value-lossless relayout/quantize only):
  - adj shard -> fp8e4 in [p, ck, c, i] layout: each chunk's DMA source
    is contiguous per partition (16 KB runs -> few descriptors).
  - x is staged twice: row-blocks 4..7 of each chunk as fp16
    (standard matmuls), row-blocks 0..3 as fp8e4 hi-precision-enough
    PAIRS [p, ck, pair, slot, d] for DoubleRow matmuls.
  - U^T fp16.

Per-core kernel v4 (improvements over v3):
  - HYBRID agg: half the contraction row-blocks run as fp8 DoubleRow
    pairs (2 j-rows per PE cycle, adjacency pairs read via a dim1-stride
    AP from the same fp8 tile), half as fp16 standard. Quantizing half
    of x to fp8e4 costs ~1.6e-2 rel err (measured 2.3e-2 for all-fp8,
    scales with sqrt of the fp8 fraction) -- inside the 2e-2 budget,
    and cuts agg PE time by ~25%.
  - deg runs ONE CHUNK AHEAD of agg (iter k: agg(k) then deg(k+1)),
    so deg completes ~1 chunk before the agg stream ends and the
    reciprocal -> broadcast-matmul chain fully hides under agg(15);
    the old version serialized ~13 us of tail and re-throttled HAM.
  - reciprocal via reciprocal_approx_fast (fp32, ~0.7us) + fp16 copy
    instead of the 3.4us exact reciprocal.
  - startup: x arrives in per-2-chunk slices interleaved with the
    adjacency chunks instead of 4 MB upfront; chunk 0 lands in four
    0.5 MB pieces so the PE starts at ~13us instead of ~22us. Warmup
    train sized to bridge exactly to the first chunk.

fp8 is exact for the adjacency and deg; the finale uses an exact
broadcast matmul (1/32-stationary over 32 replicas) to spread 1/deg to
all partitions. All accumulation is fp32 in PSUM.
"""

import sys

if "/opt/trn_rl_repo" not in sys.path:
    sys.path.insert(0, "/opt/trn_rl_repo")

import numpy as np

from concourse import bacc, mybir, tile
from concourse.bass import ts
from concourse.bass_utils import run_bass_kernel_spmd

N = 16384  # nodes
D = 128  # features
CORES = 8
S = N // CORES  # 2048 output nodes per core
P = 128  # partitions
JB = N // P  # 128 row-blocks
IC = S // 512  # 4 moving-dim chunks of 512
CH = 8  # row-blocks per adj chunk
NCK = JB // CH  # 16 chunks
NPAIR = 2  # DoubleRow pairs per chunk (covers row-blocks 0..2*NPAIR-1)
NF16 = CH - 2 * NPAIR  # fp16 row-blocks per chunk (row-blocks 2*NPAIR..)
WARM_MMS = 10

F16 = mybir.dt.float16
F32 = mybir.dt.float32
F8 = mybir.dt.float8e4
DR = mybir.MatmulPerfMode.DoubleRow


def build_nc():
    nc = bacc.Bacc("TRN2", target_bir_lowering=False, debug=False)

    a_dram = nc.dram_tensor("a", [P, JB * S], F8, kind="ExternalInput").ap()
    x16_dram = nc.dram_tensor(
        "x16", [P, NCK * NF16 * D], F16, kind="ExternalInput"
    ).ap()
    x8_dram = nc.dram_tensor(
        "x8", [P, NCK * NPAIR * 2 * D], F8, kind="ExternalInput"
    ).ap()
    ut_dram = nc.dram_tensor("ut", [D, D], F16, kind="ExternalInput").ap()
    # [e, i] layout; host transposes to [2048, 128]
    out_dram = nc.dram_tensor("out", [D, S], F32, kind="ExternalOutput").ap()

    x16_r = x16_dram.rearrange("p (g d) -> p g d", g=NCK * NF16)
    x8_r = x8_dram.rearrange("p (q s d) -> p q s d", q=NCK * NPAIR, s=2)
    a_r = a_dram.rearrange("p (jb i) -> p jb i", jb=JB)

    with tile.TileContext(nc) as tc:
        with (
            tc.tile_pool(name="persist", bufs=1) as persist,
            tc.tile_pool(name="adj", bufs=4) as adj_pool,
        ):
            x16h = persist.tile([P, NCK * NF16, D], F16)
            x8h = persist.tile([P, NCK * NPAIR, 2, D], F8)
            ut16 = persist.tile([D, D], F16)
            # fp8 ones stationary for the M=32 col-tiled deg matmuls
            ones32 = persist.tile([P, 32], F8)
            nc.gpsimd.memset(ones32[:], 1.0)
            # 1/32 stationary: broadcasts one 32-replica group to 128
            # partitions exactly (32 * v/32 sums bit-exact in fp32)
            avg32 = persist.tile([P, P], F8)
            nc.gpsimd.memset(avg32[:], 1.0 / 32.0)
            # scratch moving operand for the PE warmup train
            warm = persist.tile([P, 512], F8)
            nc.gpsimd.memset(warm[:], 1.0)

            ag16 = persist.tile([P, S], F16)
            rdeg32 = persist.tile([P, 512], F32)
            # fp16 so the fp8 broadcast matmul accepts it as rhs
            rdeg16 = persist.tile([P, 512], F16)
            rdeg_bc = persist.tile([P, IC, 512], F32)
            o_chunks = [
                persist.tile([P, 512], F32, name=f"o_chunk{i}") for i in range(IC)
            ]

            with tc.tile_pool(name="wps", bufs=1, space="PSUM") as wps:
                ps_warm = wps.tile([P, 512], F32, name="ps_warm")
                # throwaway matmuls: keep the PE busy through DMA-queue
                # init so HAM is 8/8 when real data arrives, sized to end
                # right as chunk 0 lands
                for _ in range(WARM_MMS):
                    nc.tensor.matmul(
                        ps_warm[0:32, :], ones32[:], warm[:], start=True, stop=True
                    )

            with (
                tc.tile_pool(name="mmps", bufs=1, space="PSUM") as mmps,
                tc.tile_pool(name="bcps", bufs=1, space="PSUM") as bcps,
                tc.tile_pool(name="fps", bufs=2, space="PSUM") as fps,
            ):
                ps_agg = [mmps.tile([P, 512], F32, name=f"ps_agg{i}") for i in range(IC)]
                ps_deg = mmps.tile([P, 512], F32, name="ps_deg")

                af_tiles = {}

                # ramp: small chunks while the DMA queues fill, then
                # 8-rb chunks. deg is merged into its own chunk during
                # the ramp and runs one chunk AHEAD from AHEAD_FROM on.
                CHUNK_PLAN = [(0, 2), (2, 2), (4, 2), (6, 2), (8, 4), (12, 4)] + [
                    (16 + 8 * i, 8) for i in range(14)
                ]
                NCH = len(CHUNK_PLAN)
                AHEAD_FROM = 6

                def issue_af(ci):
                    jb0, nrb = CHUNK_PLAN[ci]
                    af = adj_pool.tile([P, CH, S], F8, tag="af")
                    af_tiles[ci] = af
                    eng = nc.sync if ci % 2 == 0 else nc.scalar
                    eng.dma_start(af[:, 0:nrb, :], a_r[:, jb0 : jb0 + nrb, :])

                def issue_x(ga, gb, eng):
                    # x slices for 8-jb groups [ga, gb)
                    eng.dma_start(
                        x8h[:, 2 * ga : 2 * gb, :, :], x8_r[:, 2 * ga : 2 * gb, :, :]
                    )
                    eng.dma_start(
                        x16h[:, 4 * ga : 4 * gb, :], x16_r[:, 4 * ga : 4 * gb, :]
                    )

                def deg_burst(ci):
                    jb0, nrb = CHUNK_PLAN[ci]
                    af = af_tiles[ci]
                    for c in range(nrb):
                        jb = jb0 + c
                        first, last = jb == 0, jb == JB - 1
                        for ic in range(IC):
                            nc.tensor.matmul(
                                ps_deg[32 * ic : 32 * ic + 32, :],
                                ones32[:],
                                af[:, c, ts(ic, 512)],
                                start=first,
                                stop=last,
                                tile_position=(0, 32 * ic),
                            )

                def dr_mm(ci, jb, ic, start):
                    jb0, _ = CHUNK_PLAN[ci]
                    q = (jb // 8) * 2 + (jb % 8) // 2
                    nc.tensor.matmul(
                        ps_agg[ic][:],
                        x8h[:, q, :, :],
                        af_tiles[ci][:, jb - jb0 : jb - jb0 + 2, ts(ic, 512)],
                        start=start,
                        stop=False,
                        perf_mode=DR,
                    )

                def f16_mm(ci, jb, ic, stop):
                    jb0, _ = CHUNK_PLAN[ci]
                    g = (jb // 8) * 4 + (jb % 8) - 4
                    nc.tensor.matmul(
                        ps_agg[ic][:],
                        x16h[:, g, :],
                        af_tiles[ci][:, jb - jb0, ts(ic, 512)],
                        start=False,
                        stop=stop,
                    )

                def agg_chunk(ci):
                    # fp16 row-blocks first: the iter-boundary stationary
                    # reload after deg is then a cheap FWL fp16 load
                    jb0, nrb = CHUNK_PLAN[ci]
                    f16s = [jb for jb in range(jb0, jb0 + nrb) if jb % 8 >= 4]
                    drs = [jb for jb in range(jb0, jb0 + nrb) if jb % 8 < 4 and jb % 2 == 0]
                    for jb in f16s:
                        for ic in range(IC):
                            f16_mm(ci, jb, ic, False)
                    for k, jb in enumerate(drs):
                        for ic in range(IC):
                            dr_mm(ci, jb, ic, ci == 0 and k == 0)

                def bc_mm(ic):
                    bc = bcps.tile([P, 512], F32, tag="bc")
                    nc.tensor.matmul(
                        bc[:],
                        avg32[32 * ic : 32 * ic + 32, :],
                        rdeg16[32 * ic : 32 * ic + 32, :],
                        start=True,
                        stop=True,
                        # auto-derive caps at 64; the 4th row strip must
                        # be explicit
                        tile_position=(32 * ic, 0),
                    )
                    nc.vector.tensor_copy(rdeg_bc[:, ic, :], bc[:])

                def finale(ic):
                    h_ps = fps.tile([P, 512], F32, tag="h")
                    nc.tensor.matmul(
                        h_ps[:], ut16[:], ag16[:, ts(ic, 512)], start=True, stop=True
                    )
                    # out = relu(h) * (1/deg) in one DVE op (relu
                    # commutes with the positive per-column scale)
                    nc.vector.scalar_tensor_tensor(
                        o_chunks[ic][:],
                        h_ps[:],
                        0.0,
                        rdeg_bc[:, ic, :],
                        mybir.AluOpType.max,
                        mybir.AluOpType.mult,
                    )
                    nc.sync.dma_start(out_dram[:, ts(ic, 512)], o_chunks[ic][:])

                # prologue: first three chunks + x for jbs 0..15
                issue_af(0)
                issue_x(0, 2, nc.scalar)
                issue_af(1)
                issue_af(2)

                for ci in range(NCH):
                    if ci + 3 <= NCH - 1:
                        issue_af(ci + 3)
                    if ci % 2 == 0 and 4 <= ci <= 16:
                        # 8-jb groups [ci-2, ci) needed from chunk ci+2 on
                        issue_x(ci - 2, ci, nc.scalar if ci % 4 == 0 else nc.sync)
                    if ci == 6:
                        nc.scalar.dma_start(ut16[:], ut_dram[:])

                    if ci < AHEAD_FROM:
                        deg_burst(ci)
                        agg_chunk(ci)
                        if ci == AHEAD_FROM - 1:
                            deg_burst(ci + 1)
                    elif ci < NCH - 1:
                        agg_chunk(ci)
                        deg_burst(ci + 1)
                        if ci + 1 == NCH - 1:
                            # deg complete: 1/deg while the rest streams
                            nc.vector.reciprocal_approx_fast(rdeg32[:], ps_deg[:])
                            nc.vector.tensor_copy(rdeg16[:], rdeg32[:])
                    else:
                        # last chunk ic-major; each bank's drain, bc,
                        # U-matmul, relu-scale and output DMA overlap the
                        # remaining agg stream
                        jb0, nrb = CHUNK_PLAN[ci]
                        for ic in range(IC):
                            for jb in (jb0, jb0 + 2):
                                dr_mm(ci, jb, ic, False)
                            for jb in range(jb0 + 4, jb0 + 8):
                                f16_mm(ci, jb, ic, jb == jb0 + 7)
                            # drain on ScalarE (DVE owns the finale ops)
                            nc.scalar.copy(ag16[:, ts(ic, 512)], ps_agg[ic][:])
                            if ic >= 1:
                                bc_mm(ic - 1)
                            if ic >= 2:
                                finale(ic - 2)
                        bc_mm(IC - 1)
                        finale(IC - 2)
                        finale(IC - 1)

    nc.compile()
    return nc


_NC = None


def _get_nc():
    global _NC
    if _NC is None:
        _NC = build_nc()
    return _NC


def prep_in_maps(x, adj_mat, U):
    import ml_dtypes

    x = np.asarray(x, dtype=np.float32)
    adj_mat = np.asarray(adj_mat)
    U = np.asarray(U, dtype=np.float32)

    # x row-blocks 4..7 of each chunk -> fp16 [p, ck, r, d]
    xr = x.reshape(NCK, CH, P, D)
    x16 = np.ascontiguousarray(
        xr[:, 2 * NPAIR :, :, :].transpose(2, 0, 1, 3).astype(np.float16)
    ).reshape(P, NCK * NF16 * D)
    # x row-blocks 0..3 as fp8 DoubleRow pairs [p, ck, pair, slot, d]
    x8p = np.ascontiguousarray(
        xr[:, : 2 * NPAIR, :, :]
        .reshape(NCK, NPAIR, 2, P, D)
        .transpose(3, 0, 1, 2, 4)
        .astype(ml_dtypes.float8_e4m3)
    ).reshape(P, NCK * NPAIR * 2 * D)
    ut = np.ascontiguousarray(U.T.astype(np.float16))
    # adjacency values are {0,1}: exact in fp8e4m3; LUT avoids a float
    # astype over 1 GiB. Layout [p, ck, c, i] so each chunk's DMA source
    # is contiguous per partition.
    lut = np.zeros(2, dtype=np.uint8)
    lut[1] = np.array(1.0, dtype=ml_dtypes.float8_e4m3).view(np.uint8)
    in_maps = []
    for c in range(CORES):
        a8 = lut[adj_mat[:, c * S : (c + 1) * S]]
        a8 = np.ascontiguousarray(
            a8.reshape(NCK, CH, P, S).transpose(2, 0, 1, 3)
        ).view(ml_dtypes.float8_e4m3).reshape(P, JB * S)
        in_maps.append({"a": a8, "x16": x16, "x8": x8p, "ut": ut})
    return in_maps


def assemble_out(results):
    # per-core out is [128, 2048] in [e, i] layout
    parts = []
    for c in range(CORES):
        parts.append(np.ascontiguousarray(results[c]["out"].T))
    return np.concatenate(parts, axis=0)[None]


def kernel(x, adj_mat, U, **_):
    nc = _get_nc()
    in_maps = prep_in_maps(x, adj_mat, U)
    res = run_bass_kernel_spmd(nc, in_maps, core_ids=list(range(CORES)))
    return assemble_out(res.results)


# revision 8
# speedup vs baseline: 1.0036x; 1.0036x over previous
"""GNN message-passing layer on 8 TRN2 NeuronCores (v4).

Reference computation (N=16384, D=128):
    a    = adj_mat.astype(f32)            # [N, N]
    deg  = a.sum(axis=0)                  # [N]
    agg  = (a^T @ x) / deg[:, None]       # [N, D]
    out  = relu(agg @ U^T)[None]          # [1, N, D]

Sharding: column-shard adj_mat across the 8 cores (core c owns output
nodes i in [c*2048, (c+1)*2048)); x and U replicated. The contraction
over j is fully local per core — no collective.

Host staging (value-lossless relayout/quantize only):
  - adj shard -> fp8e4 in [p, ck, c, i] layout: each chunk's DMA source
    is contiguous per partition (16 KB runs -> few descriptors).
  - x is staged twice: row-blocks 4..7 of each chunk as fp16
    (standard matmuls), row-blocks 0..3 as fp8e4 hi-precision-enough
    PAIRS [p, ck, pair, slot, d] for DoubleRow matmuls.
  - U^T fp16.

Per-core kernel v4 (improvements over v3):
  - HYBRID agg: half the contraction row-blocks run as fp8 DoubleRow
    pairs (2 j-rows per PE cycle, adjacency pairs read via a dim1-stride
    AP from the same fp8 tile), half as fp16 standard. Quantizing half
    of x to fp8e4 costs ~1.6e-2 rel err (measured 2.3e-2 for all-fp8,
    scales with sqrt of the fp8 fraction) -- inside the 2e-2 budget,
    and cuts agg PE time by ~25%.
  - deg runs ONE CHUNK AHEAD of agg (iter k: agg(k) then deg(k+1)),
    so deg completes ~1 chunk before the agg stream ends and the
    reciprocal -> broadcast-matmul chain fully hides under agg(15);
    the old version serialized ~13 us of tail and re-throttled HAM.
  - reciprocal via reciprocal_approx_fast (fp32, ~0.7us) + fp16 copy
    instead of the 3.4us exact reciprocal.
  - startup: x arrives in per-2-chunk slices interleaved with the
    adjacency chunks instead of 4 MB upfront; chunk 0 lands in four
    0.5 MB pieces so the PE starts at ~13us instead of ~22us. Warmup
    train sized to bridge exactly to the first chunk.

fp8 is exact for the adjacency and deg; the finale uses an exact
broadcast matmul (1/32-stationary over 32 replicas) to spread 1/deg to
all partitions. All accumulation is fp32 in PSUM.
"""

import sys

if "/opt/trn_rl_repo" not in sys.path:
    sys.path.insert(0, "/opt/trn_rl_repo")

import numpy as np

from concourse import bacc, mybir, tile
from concourse.bass import ts
from concourse.bass_utils import run_bass_kernel_spmd

N = 16384  # nodes
D = 128  # features
CORES = 8
S = N // CORES  # 2048 output nodes per core
P = 128  # partitions
JB = N // P  # 128 row-blocks
IC = S // 512  # 4 moving-dim chunks of 512
CH = 8  # row-blocks per adj chunk
NCK = JB // CH  # 16 chunks
NPAIR = 2  # DoubleRow pairs per chunk (covers row-blocks 0..2*NPAIR-1)
NF16 = CH - 2 * NPAIR  # fp16 row-blocks per chunk (row-blocks 2*NPAIR..)
WARM_MMS = 14

F16 = mybir.dt.float16
F32 = mybir.dt.float32
F8 = mybir.dt.float8e4
DR = mybir.MatmulPerfMode.DoubleRow


def build_nc():
    nc = bacc.Bacc("TRN2", target_bir_lowering=False, debug=False)

    a_dram = nc.dram_tensor("a", [P, JB * S], F8, kind="ExternalInput").ap()
    x16_dram = nc.dram_tensor(
        "x16", [P, NCK * NF16 * D], F16, kind="ExternalInput"
    ).ap()
    x8_dram = nc.dram_tensor(
        "x8", [P, NCK * NPAIR * 2 * D], F8, kind="ExternalInput"
    ).ap()
    ut_dram = nc.dram_tensor("ut", [D, D], F16, kind="ExternalInput").ap()
    # [e, i] layout; host transposes to [2048, 128]
    out_dram = nc.dram_tensor("out", [D, S], F32, kind="ExternalOutput").ap()

    x16_r = x16_dram.rearrange("p (g d) -> p g d", g=NCK * NF16)
    x8_r = x8_dram.rearrange("p (q s d) -> p q s d", q=NCK * NPAIR, s=2)
    a_r = a_dram.rearrange("p (jb i) -> p jb i", jb=JB)

    with tile.TileContext(nc) as tc:
        with (
            tc.tile_pool(name="persist", bufs=1) as persist,
            tc.tile_pool(name="adj", bufs=4) as adj_pool,
        ):
            x16h = persist.tile([P, NCK * NF16, D], F16)
            x8h = persist.tile([P, NCK * NPAIR, 2, D], F8)
            ut16 = persist.tile([D, D], F16)
            # fp8 ones stationary for the M=32 col-tiled deg matmuls
            ones32 = persist.tile([P, 32], F8)
            nc.gpsimd.memset(ones32[:], 1.0)
            # 1/32 stationary: broadcasts one 32-replica group to 128
            # partitions exactly (32 * v/32 sums bit-exact in fp32)
            avg32 = persist.tile([P, P], F8)
            nc.gpsimd.memset(avg32[:], 1.0 / 32.0)
            # scratch moving operand for the PE warmup train
            warm = persist.tile([P, 512], F8)
            nc.gpsimd.memset(warm[:], 1.0)

            ag16 = persist.tile([P, S], F16)
            rdeg32 = persist.tile([P, 512], F32)
            # fp16 so the fp8 broadcast matmul accepts it as rhs
            rdeg16 = persist.tile([P, 512], F16)
            rdeg_bc = persist.tile([P, IC, 512], F32)
            o_chunks = [
                persist.tile([P, 512], F32, name=f"o_chunk{i}") for i in range(IC)
            ]

            with tc.tile_pool(name="wps", bufs=1, space="PSUM") as wps:
                ps_warm = wps.tile([P, 512], F32, name="ps_warm")
                # throwaway matmuls: keep the PE busy through DMA-queue
                # init so HAM is 8/8 when real data arrives, sized to end
                # right as chunk 0 lands
                for _ in range(WARM_MMS):
                    nc.tensor.matmul(
                        ps_warm[0:32, :], ones32[:], warm[:], start=True, stop=True
                    )

            with (
                tc.tile_pool(name="mmps", bufs=1, space="PSUM") as mmps,
                tc.tile_pool(name="bcps", bufs=1, space="PSUM") as bcps,
                tc.tile_pool(name="fps", bufs=2, space="PSUM") as fps,
            ):
                ps_agg = [mmps.tile([P, 512], F32, name=f"ps_agg{i}") for i in range(IC)]
                ps_deg = mmps.tile([P, 512], F32, name="ps_deg")

                af_tiles = {}

                # ramp: small chunks while the DMA queues fill, then
                # 8-rb chunks. deg is merged into its own chunk during
                # the ramp and runs one chunk AHEAD from AHEAD_FROM on.
                CHUNK_PLAN = (
                    [(0, 2), (2, 2), (4, 2), (6, 2)]
                    + [(8 + 4 * i, 4) for i in range(6)]
                    + [(32 + 8 * i, 8) for i in range(12)]
                )
                NCH = len(CHUNK_PLAN)
                AHEAD_FROM = 10
                # filler matmuls after early chunks: absorb sub-window
                # DMA stalls without letting HAM re-throttle
                FILLERS = {2: 2, 3: 2, 4: 3, 5: 3, 6: 3, 7: 3, 8: 4, 9: 4, 10: 3, 11: 2}

                def issue_af(ci):
                    jb0, nrb = CHUNK_PLAN[ci]
                    af = adj_pool.tile([P, CH, S], F8, tag="af")
                    af_tiles[ci] = af
                    eng = nc.sync if ci % 2 == 0 else nc.scalar
                    eng.dma_start(af[:, 0:nrb, :], a_r[:, jb0 : jb0 + nrb, :])

                def issue_x(ga, gb, eng):
                    # x slices for 8-jb groups [ga, gb)
                    eng.dma_start(
                        x8h[:, 2 * ga : 2 * gb, :, :], x8_r[:, 2 * ga : 2 * gb, :, :]
                    )
                    eng.dma_start(
                        x16h[:, 4 * ga : 4 * gb, :], x16_r[:, 4 * ga : 4 * gb, :]
                    )

                def deg_burst(ci):
                    jb0, nrb = CHUNK_PLAN[ci]
                    af = af_tiles[ci]
                    for c in range(nrb):
                        jb = jb0 + c
                        first, last = jb == 0, jb == JB - 1
                        for ic in range(IC):
                            nc.tensor.matmul(
                                ps_deg[32 * ic : 32 * ic + 32, :],
                                ones32[:],
                                af[:, c, ts(ic, 512)],
                                start=first,
                                stop=last,
                                tile_position=(0, 32 * ic),
                            )

                def dr_mm(ci, jb, ic, start):
                    jb0, _ = CHUNK_PLAN[ci]
                    q = (jb // 8) * 2 + (jb % 8) // 2
                    nc.tensor.matmul(
                        ps_agg[ic][:],
                        x8h[:, q, :, :],
                        af_tiles[ci][:, jb - jb0 : jb - jb0 + 2, ts(ic, 512)],
                        start=start,
                        stop=False,
                        perf_mode=DR,
                    )

                def f16_mm(ci, jb, ic, stop):
                    jb0, _ = CHUNK_PLAN[ci]
                    g = (jb // 8) * 4 + (jb % 8) - 4
                    nc.tensor.matmul(
                        ps_agg[ic][:],
                        x16h[:, g, :],
                        af_tiles[ci][:, jb - jb0, ts(ic, 512)],
                        start=False,
                        stop=stop,
                    )

                def agg_chunk(ci):
                    # fp16 row-blocks first: the iter-boundary stationary
                    # reload after deg is then a cheap FWL fp16 load
                    jb0, nrb = CHUNK_PLAN[ci]
                    f16s = [jb for jb in range(jb0, jb0 + nrb) if jb % 8 >= 4]
                    drs = [jb for jb in range(jb0, jb0 + nrb) if jb % 8 < 4 and jb % 2 == 0]
                    for jb in f16s:
                        for ic in range(IC):
                            f16_mm(ci, jb, ic, False)
                    for k, jb in enumerate(drs):
                        for ic in range(IC):
                            dr_mm(ci, jb, ic, ci == 0 and k == 0)

                def filler(n):
                    # warm-keeping matmuls into the (otherwise idle until
                    # the tail) bc bank; full-overwrite groups
                    fl = bcps.tile([P, 512], F32, tag="bc")
                    for _ in range(n):
                        nc.tensor.matmul(
                            fl[0:32, :], ones32[:], warm[:], start=True, stop=True
                        )

                def bc_mm(ic):
                    bc = bcps.tile([P, 512], F32, tag="bc")
                    nc.tensor.matmul(
                        bc[:],
                        avg32[32 * ic : 32 * ic + 32, :],
                        rdeg16[32 * ic : 32 * ic + 32, :],
                        start=True,
                        stop=True,
                        # auto-derive caps at 64; the 4th row strip must
                        # be explicit
                        tile_position=(32 * ic, 0),
                    )
                    nc.vector.tensor_copy(rdeg_bc[:, ic, :], bc[:])

                def finale(ic):
                    h_ps = fps.tile([P, 512], F32, tag="h")
                    nc.tensor.matmul(
                        h_ps[:], ut16[:], ag16[:, ts(ic, 512)], start=True, stop=True
                    )
                    # out = relu(h) * (1/deg) in one DVE op (relu
                    # commutes with the positive per-column scale)
                    nc.vector.scalar_tensor_tensor(
                        o_chunks[ic][:],
                        h_ps[:],
                        0.0,
                        rdeg_bc[:, ic, :],
                        mybir.AluOpType.max,
                        mybir.AluOpType.mult,
                    )
                    nc.sync.dma_start(out_dram[:, ts(ic, 512)], o_chunks[ic][:])

                # prologue: first three chunks + x for jbs 0..15
                issue_af(0)
                issue_x(0, 2, nc.scalar)
                issue_af(1)
                issue_af(2)

                for ci in range(NCH):
                    if ci + 3 <= NCH - 1:
                        issue_af(ci + 3)
                    if ci % 2 == 0 and 4 <= ci <= 16:
                        # 8-jb groups [ci-2, ci) needed from chunk ~ci+2 on
                        issue_x(ci - 2, ci, nc.scalar if ci % 4 == 0 else nc.sync)
                    if ci == 6:
                        nc.scalar.dma_start(ut16[:], ut_dram[:])

                    if ci < AHEAD_FROM:
                        deg_burst(ci)
                        agg_chunk(ci)
                        if ci == AHEAD_FROM - 1:
                            deg_burst(ci + 1)
                        if ci in FILLERS:
                            filler(FILLERS[ci])
                    elif ci < NCH - 1:
                        agg_chunk(ci)
                        deg_burst(ci + 1)
                        if ci in FILLERS:
                            filler(FILLERS[ci])
                        if ci + 1 == NCH - 1:
                            # deg complete: 1/deg while the rest streams
                            nc.vector.reciprocal_approx_fast(rdeg32[:], ps_deg[:])
                            nc.vector.tensor_copy(rdeg16[:], rdeg32[:])
                    else:
                        # last chunk ic-major; each bank's drain, bc,
                        # U-matmul, relu-scale and output DMA overlap the
                        # remaining agg stream
                        jb0, nrb = CHUNK_PLAN[ci]
                        for ic in range(IC):
                            for jb in (jb0, jb0 + 2):
                                dr_mm(ci, jb, ic, False)
                            for jb in range(jb0 + 4, jb0 + 8):
                                f16_mm(ci, jb, ic, jb == jb0 + 7)
                            # drain on ScalarE (DVE owns the finale ops);
                            # the last ic drains at half granularity below
                            if ic < IC - 1:
                                nc.scalar.copy(ag16[:, ts(ic, 512)], ps_agg[ic][:])
                            if ic >= 1:
                                bc_mm(ic - 1)
                            if ic >= 2:
                                finale(ic - 2)
                        bc_mm(IC - 1)
                        finale(IC - 2)
                        # last chunk of output at half granularity so the
                        # drain/matmul/relu-scale/DMA chain pipelines
                        h_ps = fps.tile([P, 512], F32, tag="h")
                        for h in range(2):
                            sl = slice((IC - 1) * 512 + 256 * h, (IC - 1) * 512 + 256 * h + 256)
                            nc.scalar.copy(ag16[:, sl], ps_agg[IC - 1][:, 256 * h : 256 * h + 256])
                            nc.tensor.matmul(
                                h_ps[:, 256 * h : 256 * h + 256],
                                ut16[:],
                                ag16[:, sl],
                                start=True,
                                stop=True,
                            )
                            nc.vector.scalar_tensor_tensor(
                                o_chunks[IC - 1][:, 256 * h : 256 * h + 256],
                                h_ps[:, 256 * h : 256 * h + 256],
                                0.0,
                                rdeg_bc[:, IC - 1, 256 * h : 256 * h + 256],
                                mybir.AluOpType.max,
                                mybir.AluOpType.mult,
                            )
                            nc.sync.dma_start(
                                out_dram[:, sl], o_chunks[IC - 1][:, 256 * h : 256 * h + 256]
                            )

    nc.compile()
    return nc


_NC = None


def _get_nc():
    global _NC
    if _NC is None:
        _NC = build_nc()
    return _NC


def prep_in_maps(x, adj_mat, U):
    import ml_dtypes

    x = np.asarray(x, dtype=np.float32)
    adj_mat = np.asarray(adj_mat)
    U = np.asarray(U, dtype=np.float32)

    # x row-blocks 4..7 of each chunk -> fp16 [p, ck, r, d]
    xr = x.reshape(NCK, CH, P, D)
    x16 = np.ascontiguousarray(
        xr[:, 2 * NPAIR :, :, :].transpose(2, 0, 1, 3).astype(np.float16)
    ).reshape(P, NCK * NF16 * D)
    # x row-blocks 0..3 as fp8 DoubleRow pairs [p, ck, pair, slot, d]
    x8p = np.ascontiguousarray(
        xr[:, : 2 * NPAIR, :, :]
        .reshape(NCK, NPAIR, 2, P, D)
        .transpose(3, 0, 1, 2, 4)
        .astype(ml_dtypes.float8_e4m3)
    ).reshape(P, NCK * NPAIR * 2 * D)
    ut = np.ascontiguousarray(U.T.astype(np.float16))
    # adjacency values are {0,1}: exact in fp8e4m3; LUT avoids a float
    # astype over 1 GiB. Layout [p, ck, c, i] so each chunk's DMA source
    # is contiguous per partition.
    lut = np.zeros(2, dtype=np.uint8)
    lut[1] = np.array(1.0, dtype=ml_dtypes.float8_e4m3).view(np.uint8)
    in_maps = []
    for c in range(CORES):
        a8 = lut[adj_mat[:, c * S : (c + 1) * S]]
        a8 = np.ascontiguousarray(
            a8.reshape(NCK, CH, P, S).transpose(2, 0, 1, 3)
        ).view(ml_dtypes.float8_e4m3).reshape(P, JB * S)
        in_maps.append({"a": a8, "x16": x16, "x8": x8p, "ut": ut})
    return in_maps


def assemble_out(results):
    # per-core out is [128, 2048] in [e, i] layout
    parts = []
    for c in range(CORES):
        parts.append(np.ascontiguousarray(results[c]["out"].T))
    return np.concatenate(parts, axis=0)[None]


def kernel(x, adj_mat, U, **_):
    nc = _get_nc()
    in_maps = prep_in_maps(x, adj_mat, U)
    res = run_bass_kernel_spmd(nc, in_maps, core_ids=list(range(CORES)))
    return assemble_out(res.results)
